# revision 1
# baseline (speedup 1.0000x reference)
"""Trainium2 Bass kernel for nn_ClosedFormLoss (closed-form matting Laplacian loss).

Math: the reference computes, per (batch, class), y = keep * (vals @ ow) per 3x3
window followed by a scatter-add, where vals is the 9x9 matting-Laplacian block
built from a per-window 3x3 color covariance inverse.  Expanded per window w and
pixel p = w+k:
    (vals @ ow)_k = ow_k - (1/9) * (S(w) + (imgn(p) - mu(w)) . v(w))
with S = box3(o), q_ch = box3(imgn_ch * o), t = q - mu*S, v = inv * t.
The scatter-add (windows -> pixels) is the transpose box filter, so
    Lo(p) = o(p)*Nk(p) - A(p) - sum_ch imgn_ch(p) * B_ch(p)
    A  = box3T(keep/9 * (S - mu.v)),  B_ch = box3T(keep/9 * v_ch),
    Nk = box3T(keep)
and loss_bc = sum_p (Lo + conf*o - tri*conf)^2 / n^2,  n = H*W.

All box filters run on the tensor engine as banded-matrix matmuls (partition dim)
accumulated over 3 column shifts (free dim).  Everything else is elementwise on
DVE/ACT.  Sharding: 8 cores = 2 batches x 4 row-quarters; each core processes its
quarter as two 64/65-row sub-stripes and emits one partial sum of squares.
"""

import sys
import numpy as np

sys.path.insert(0, "/opt/trn_rl_repo")

from concourse import bacc, mybir, tile  # noqa: E402
from concourse.bass_utils import run_bass_kernel_spmd  # noqa: E402

F32 = mybir.dt.float32
BF16 = mybir.dt.bfloat16
I32 = mybir.dt.int32
OP = mybir.AluOpType

N_CLASSES = 7
H = W = 513
NWC = 511          # window grid cols
N = H * W
EPS9 = 1e-7 / 9.0
TRI_CONF = 100.0
# keep9 is stored in bf16; 9_eff undoes the bf16 rounding of 1/9 exactly
import ml_dtypes  # noqa: E402
NINE_EFF = float(1.0 / np.float32(np.float32(1.0 / 9.0).astype(ml_dtypes.bfloat16)))

# 8 sub-stripes per batch image; sub s owns output pixel rows [64s, 64s+64)
# (sub 7 owns [448, 513)).  Each core handles one batch and two subs.
SUB_R0 = [64 * s for s in range(8)]
N_OUT = 65          # rows computed per sub (65 with 1-row overlap)
N_WIN = 67          # window rows per sub: [r0-2, r0+65)
N_PIX = 69          # pixel rows for box inputs: [r0-2, r0+67)
N_M = 71            # trimap rows for dilate: [r0-3, r0+68)
# local row l of every pixel/window-aligned tile <-> global row r0-2+l
# output pixel rows live at local rows [2, 67)
CH0, CH1 = 257, 256  # column chunks for 513-wide transpose-box outputs

_PROGRAM = None


def _build_program():
    nc = bacc.Bacc("TRN2", target_bir_lowering=False, debug=False, num_devices=8)

    cpr_d = nc.declare_dram_parameter("cpr", [2, N_CLASSES, N_PIX, W], F32, isOutput=False)
    img_d = nc.declare_dram_parameter("img", [2, 3, N_PIX, W], F32, isOutput=False)
    trim_d = nc.declare_dram_parameter("trim", [2, N_M, W], I32, isOutput=False)
    band_d = nc.declare_dram_parameter("band", [N_M, N_PIX], F32, isOutput=False)
    vmask_d = nc.declare_dram_parameter("vmask9", [2, N_WIN, 1], F32, isOutput=False)
    rmask_d = nc.declare_dram_parameter("rmask", [2, N_OUT, 1], F32, isOutput=False)
    ones_d = nc.declare_dram_parameter("ones", [N_OUT, 1], F32, isOutput=False)
    part_d = nc.declare_dram_parameter("partial", [1, 1], F32, isOutput=True)

    with tile.TileContext(nc) as tc:
        with (
            tc.tile_pool(name="sb", bufs=1) as sb,
            tc.tile_pool(name="spc", bufs=3) as spc,
            tc.tile_pool(name="sc2", bufs=3) as sc2,
            tc.tile_pool(name="sc1", bufs=1) as sc1,
            tc.tile_pool(name="cst", bufs=1) as cst,
            tc.tile_pool(name="psf", bufs=4, space="PSUM") as psf,
            tc.tile_pool(name="psb", bufs=4, space="PSUM") as psb,
        ):
            band = cst.tile([N_M, N_PIX], F32, name="band", tag="band")
            band_bf = cst.tile([N_M, N_PIX], BF16, name="band_bf", tag="band_bf")
            ones = cst.tile([N_OUT, 1], F32, name="ones", tag="ones")
            nc.sync.dma_start(band[:], band_d[:])
            nc.sync.dma_start(ones[:], ones_d[:])
            nc.vector.tensor_copy(band_bf[:], band[:])

            def fwd_box(dst_ps, src, wm=None, nk=N_PIX, nm=N_WIN):
                # dst[m, j] = sum_{dj} sum_k band[k, m] * src[k, j+dj]
                wm = band if wm is None else wm
                for dj in range(3):
                    nc.tensor.matmul(
                        dst_ps[0:nm, :], wm[0:nk, 0:nm], src[0:nk, dj:dj + NWC],
                        start=(dj == 0), stop=(dj == 2),
                    )

            def t_box(dst_ps, src, c0, c1, wm=None):
                # transpose box: out pixel col j <- window cols j-2, j-1, j
                # out rows [0, 65) <-> global pixel rows [r0, r0+65)
                wm = band if wm is None else wm
                first = True
                for djw in range(3):
                    jlo, jhi = max(c0, djw), min(c1, NWC + djw)
                    nc.tensor.matmul(
                        dst_ps[0:N_OUT, jlo - c0:jhi - c0],
                        wm[0:N_WIN, 0:N_OUT],
                        src[0:N_WIN, jlo - djw:jhi - djw],
                        start=first, stop=(djw == 2),
                    )
                    first = False

            def dil_box(dst_ps, src, c0, c1, wm=None):
                # 3x3 SAME-pad dilate numerator on the pixel grid
                wm = band if wm is None else wm
                first = True
                for dj in (-1, 0, 1):
                    jlo, jhi = max(c0, -dj), min(c1, W - dj)
                    nc.tensor.matmul(
                        dst_ps[0:N_PIX, jlo - c0:jhi - c0],
                        wm[0:N_M, 0:N_PIX],
                        src[0:N_M, jlo + dj:jhi + dj],
                        start=first, stop=(dj == 1),
                    )
                    first = False

            accm_tot = cst.tile([N_OUT, 1], F32, name="accm_tot", tag="accm_tot")

            for s in range(2):
                # ---- load per-sub inputs ----
                img_raw = [sb.tile([N_PIX, W], F32, name=f"imgr{ch}", tag=f"imgr{ch}") for ch in range(3)]
                for ch in range(3):
                    nc.sync.dma_start(img_raw[ch][:], img_d[s, ch])
                ti_a = sb.tile([N_M, W], I32, name="ti_a", tag="ti_a")
                nc.sync.dma_start(ti_a[:], trim_d[s])
                vmask9 = sb.tile([N_WIN, 1], F32, name="vmask9", tag="vmask9")
                rmask = sb.tile([N_OUT, 1], F32, name="rmask", tag="rmask")
                nc.sync.dma_start(vmask9[:], vmask_d[s])
                nc.sync.dma_start(rmask[:], rmask_d[s])

                # ---- per-batch precompute ----
                imgn = [sb.tile([N_PIX, W], F32, name=f"imgn{ch}", tag=f"imgn{ch}") for ch in range(3)]
                for ch in range(3):
                    nc.vector.tensor_scalar(imgn[ch][:], img_raw[ch][:], 1.0 / 255.0, None, OP.mult)
                tf_a = sb.tile([N_M, W], F32, name="tf_a", tag="tf_a")
                nc.vector.tensor_copy(tf_a[:], ti_a[:])
                # out-aligned copies (local row l <-> global pixel row r0+l)
                img_out_raw = [sb.tile([N_OUT, W], F32, name=f"imgor{ch}", tag=f"imgor{ch}")
                               for ch in range(3)]
                imgn_out = [sb.tile([N_OUT, W], F32, name=f"imgno{ch}", tag=f"imgno{ch}")
                            for ch in range(3)]
                for ch in range(3):
                    nc.sync.dma_start(img_out_raw[ch][:], img_d[s, ch, 2:2 + N_OUT])
                    nc.vector.tensor_scalar(imgn_out[ch][:], img_out_raw[ch][:],
                                            1.0 / 255.0, None, OP.mult)
                ti_o = sb.tile([N_OUT, W], I32, name="ti_o", tag="ti_o")
                nc.sync.dma_start(ti_o[:], trim_d[s, 3:3 + N_OUT])
                tf_o = sb.tile([N_OUT, W], F32, name="tf_o", tag="tf_o")
                nc.vector.tensor_copy(tf_o[:], ti_o[:])
                m100o = sb.tile([N_OUT, W], F32, name="m100o", tag="m100o")
                nc.vector.tensor_scalar(m100o[:], tf_o[:], 128.0, TRI_CONF, OP.is_equal, OP.mult)
                mdil = sb.tile([N_M, W], BF16, name="mdil", tag="mdil")
                nc.vector.tensor_scalar(mdil[:], tf_a[:], 128.0, None, OP.is_equal)

                # bf16 copies for the class pipeline
                imgn_bf = [sb.tile([N_PIX, W], BF16, name=f"imgb{ch}", tag=f"imgb{ch}")
                           for ch in range(3)]
                for ch in range(3):
                    nc.scalar.activation(imgn_bf[ch][:], imgn[ch][:],
                                         mybir.ActivationFunctionType.Copy, bias=0.0, scale=1.0)

                # color moments -> mu, E2(+eps on diag); var path stays fp32/exact
                pairs = [(0, 0), (0, 1), (0, 2), (1, 1), (1, 2), (2, 2)]
                mu = [sb.tile([N_WIN, NWC], F32, name=f"mu{ch}", tag=f"mu{ch}") for ch in range(3)]
                mu_bf = [sb.tile([N_WIN, NWC], BF16, name=f"mub{ch}", tag=f"mub{ch}") for ch in range(3)]
                e2 = [sc1.tile([N_WIN, NWC], F32, name=f"e2{i}", tag=f"e2{i}") for i in range(6)]
                for ch in range(3):
                    bps = psf.tile([N_WIN, NWC], F32, name="fwd", tag="fwd")
                    fwd_box(bps, imgn[ch])
                    nc.scalar.activation(mu[ch][:], bps[:], mybir.ActivationFunctionType.Copy,
                                         bias=0.0, scale=1.0 / 9.0)
                    nc.scalar.activation(mu_bf[ch][:], bps[:], mybir.ActivationFunctionType.Copy,
                                         bias=0.0, scale=1.0 / 9.0)
                for i, (a, b) in enumerate(pairs):
                    prod = sc1.tile([N_PIX, W], F32, name="prod", tag="prod")
                    nc.gpsimd.tensor_tensor(prod[:], imgn[a][:], imgn[b][:], OP.mult)
                    bps = psf.tile([N_WIN, NWC], F32, name="fwd", tag="fwd")
                    fwd_box(bps, prod)
                    if a == b:
                        nc.vector.tensor_scalar(e2[i][:], bps[:], 1.0 / 9.0, EPS9, OP.mult, OP.add)
                    else:
                        nc.vector.tensor_scalar(e2[i][:], bps[:], 1.0 / 9.0, None, OP.mult)

                # var = E2 - mu mu^T  (6 unique entries)
                var = [sc1.tile([N_WIN, NWC], F32, name=f"var{i}", tag=f"var{i}") for i in range(6)]
                for i, (a, b) in enumerate(pairs):
                    mm = sc1.tile([N_WIN, NWC], F32, name="mm_sc", tag="mm_sc")
                    nc.gpsimd.tensor_tensor(mm[:], mu[a][:], mu[b][:], OP.mult)
                    nc.gpsimd.tensor_tensor(var[i][:], e2[i][:], mm[:], OP.subtract)
                v11, v12, v13, v22, v23, v33 = var

                # adjugate & inverse
                def fma_sub(x1, y1, x2, y2, tag):
                    # returns x1*y1 - x2*y2
                    p1 = sc1.tile([N_WIN, NWC], F32, name="cof_p1", tag="cof_p1")
                    p2 = sc1.tile([N_WIN, NWC], F32, name="cof_p2", tag="cof_p2")
                    o = sc1.tile([N_WIN, NWC], F32, name=tag, tag=tag)
                    nc.gpsimd.tensor_tensor(p1[:], x1[:], y1[:], OP.mult)
                    nc.gpsimd.tensor_tensor(p2[:], x2[:], y2[:], OP.mult)
                    nc.gpsimd.tensor_tensor(o[:], p1[:], p2[:], OP.subtract)
                    return o

                a11 = fma_sub(v22, v33, v23, v23, "a11")
                a12 = fma_sub(v13, v23, v12, v33, "a12")
                a13 = fma_sub(v12, v23, v13, v22, "a13")
                a22 = fma_sub(v11, v33, v13, v13, "a22")
                a23 = fma_sub(v12, v13, v11, v23, "a23")
                a33 = fma_sub(v11, v22, v12, v12, "a33")
                # det = v11*a11 + v12*a12 + v13*a13
                d1 = sc1.tile([N_WIN, NWC], F32, name="d1", tag="d1")
                d2 = sc1.tile([N_WIN, NWC], F32, name="d2", tag="d2")
                nc.gpsimd.tensor_tensor(d1[:], v11[:], a11[:], OP.mult)
                nc.gpsimd.tensor_tensor(d2[:], v12[:], a12[:], OP.mult)
                nc.gpsimd.tensor_tensor(d1[:], d1[:], d2[:], OP.add)
                nc.gpsimd.tensor_tensor(d2[:], v13[:], a13[:], OP.mult)
                nc.gpsimd.tensor_tensor(d1[:], d1[:], d2[:], OP.add)
                rdet = sc1.tile([N_WIN, NWC], F32, name="rdet", tag="rdet")
                nc.vector.reciprocal(rdet[:], d1[:])
                inv = [sb.tile([N_WIN, NWC], BF16, name=f"inv{i}", tag=f"inv{i}") for i in range(6)]
                for i, adj in enumerate([a11, a12, a13, a22, a23, a33]):
                    nc.vector.tensor_mul(inv[i][:], adj[:], rdet[:])
                i11, i12, i13, i22, i23, i33 = inv

                # keep mask: dilate(~consts) then window-any, then valid/9
                d01 = sb.tile([N_PIX, W], BF16, name="d01", tag="d01")
                for (c0, c1) in ((0, CH0), (CH0, W)):
                    dps = psb.tile([N_PIX, CH0], F32, name="bt", tag="bt")
                    dil_box(dps, mdil, c0, c1, wm=band_bf)
                    nc.vector.tensor_scalar(d01[:, c0:c1], dps[0:N_PIX, 0:c1 - c0], 0.0, None, OP.is_gt)
                keep9 = sb.tile([N_WIN, NWC], BF16, name="keep9", tag="keep9")
                kps = psf.tile([N_WIN, NWC], F32, name="fwd", tag="fwd")
                fwd_box(kps, d01, wm=band_bf)
                nc.vector.tensor_scalar(keep9[:], kps[:], 0.0, vmask9[:], OP.is_gt, OP.mult)

                # Nkc = 9 * box3T(keep9) + 100 - 100*m  (at output pixel rows)
                nkc = sb.tile([N_OUT, W], F32, name="nkc", tag="nkc")
                for (c0, c1) in ((0, CH0), (CH0, W)):
                    nps = psb.tile([N_PIX, CH0], F32, name="bt", tag="bt")
                    t_box(nps, keep9, c0, c1, wm=band_bf)
                    nc.vector.tensor_scalar(nkc[:, c0:c1], nps[0:N_OUT, 0:c1 - c0],
                                            NINE_EFF, TRI_CONF, OP.mult, OP.add)
                nc.vector.tensor_sub(nkc[:], nkc[:], m100o[:])

                acc_w = sb.tile([N_OUT, 16], F32, name="acc_w", tag="acc_w")

                # ---- per-class ----
                for c in range(N_CLASSES):
                    o = spc.tile([N_PIX, W], F32, name="o", tag="o")
                    nc.sync.dma_start(o[:], cpr_d[s, c])
                    o_out = spc.tile([N_OUT, W], F32, name="o_out", tag="o_out")
                    nc.sync.dma_start(o_out[:], cpr_d[s, c, 2:2 + N_OUT])
                    o_bf = spc.tile([N_PIX, W], BF16, name="o_bf", tag="o_bf")
                    nc.scalar.activation(o_bf[:], o[:],
                                         mybir.ActivationFunctionType.Copy, bias=0.0, scale=1.0)

                    sps = psf.tile([N_WIN, NWC], F32, name="fwd", tag="fwd")
                    fwd_box(sps, o_bf, wm=band_bf)
                    qps = []
                    for ch in range(3):
                        po = sc2.tile([N_PIX, W], BF16, name="po", tag="po")
                        nc.vector.tensor_mul(po[:], imgn_bf[ch][:], o_bf[:])
                        qp = psf.tile([N_WIN, NWC], F32, name="fwd", tag="fwd")
                        fwd_box(qp, po, wm=band_bf)
                        qps.append(qp)

                    # bf16 S/q in SBUF (ScalarE copies off PSUM)
                    s_bf = sc2.tile([N_WIN, NWC], BF16, name="s_bf", tag="s_bf")
                    nc.scalar.activation(s_bf[:], sps[:],
                                         mybir.ActivationFunctionType.Copy, bias=0.0, scale=1.0)
                    q_bf = []
                    for ch in range(3):
                        qb = sc2.tile([N_WIN, NWC], BF16, name=f"qb{ch}", tag=f"qb{ch}")
                        nc.scalar.activation(qb[:], qps[ch][:],
                                             mybir.ActivationFunctionType.Copy, bias=0.0, scale=1.0)
                        q_bf.append(qb)

                    # t_ch = q_ch - mu_ch * S
                    t = []
                    for ch in range(3):
                        ms = sc2.tile([N_WIN, NWC], BF16, name="ms", tag="ms")
                        nc.vector.tensor_mul(ms[:], mu_bf[ch][:], s_bf[:])
                        tt = sc2.tile([N_WIN, NWC], BF16, name=f"t{ch}", tag=f"t{ch}")
                        nc.vector.tensor_sub(tt[:], q_bf[ch][:], ms[:])
                        t.append(tt)

                    # v = inv @ t (symmetric)
                    v = []
                    for (ia, ib, ic) in ((i11, i12, i13), (i12, i22, i23), (i13, i23, i33)):
                        vv = sc2.tile([N_WIN, NWC], BF16, name="v_comp", tag="v_comp")
                        p2 = sc2.tile([N_WIN, NWC], BF16, name="v_p2", tag="v_p2")
                        nc.vector.tensor_mul(vv[:], ia[:], t[0][:])
                        nc.vector.tensor_mul(p2[:], ib[:], t[1][:])
                        nc.vector.tensor_add(vv[:], vv[:], p2[:])
                        nc.vector.tensor_mul(p2[:], ic[:], t[2][:])
                        nc.vector.tensor_add(vv[:], vv[:], p2[:])
                        v.append(vv)

                    # muv = mu . v ; ak = (S - muv) * keep9 ; bk_ch = v_ch * keep9
                    muv = sc2.tile([N_WIN, NWC], BF16, name="muv", tag="muv")
                    p2 = sc2.tile([N_WIN, NWC], BF16, name="muv_p2", tag="muv_p2")
                    nc.gpsimd.tensor_tensor(muv[:], mu_bf[0][:], v[0][:], OP.mult)
                    nc.gpsimd.tensor_tensor(p2[:], mu_bf[1][:], v[1][:], OP.mult)
                    nc.gpsimd.tensor_tensor(muv[:], muv[:], p2[:], OP.add)
                    nc.gpsimd.tensor_tensor(p2[:], mu_bf[2][:], v[2][:], OP.mult)
                    nc.gpsimd.tensor_tensor(muv[:], muv[:], p2[:], OP.add)
                    ak = sc2.tile([N_WIN, NWC], BF16, name="ak", tag="ak")
                    nc.vector.tensor_sub(ak[:], s_bf[:], muv[:])
                    nc.vector.tensor_mul(ak[:], ak[:], keep9[:])
                    bk = []
                    for ch in range(3):
                        bb = sc2.tile([N_WIN, NWC], BF16, name=f"bk{ch}", tag=f"bk{ch}")
                        nc.vector.tensor_mul(bb[:], v[ch][:], keep9[:])
                        bk.append(bb)

                    # Res = o*Nkc - A - sum imgn*B - 100*(trimap==c+1); square+reduce
                    for ci, (c0, c1) in enumerate(((0, CH0), (CH0, W))):
                        cw = c1 - c0
                        aps = psb.tile([N_PIX, CH0], F32, name="bt", tag="bt")
                        t_box(aps, ak, c0, c1, wm=band_bf)
                        bps3 = []
                        for ch in range(3):
                            bp = psb.tile([N_PIX, CH0], F32, name="bt", tag="bt")
                            t_box(bp, bk[ch], c0, c1, wm=band_bf)
                            bps3.append(bp)
                        r = sc2.tile([N_OUT, CH0], F32, name="res", tag="res")
                        p = sc2.tile([N_OUT, CH0], F32, name="res_p", tag="res_p")
                        nc.vector.tensor_mul(r[:, 0:cw], o_out[:, c0:c1], nkc[:, c0:c1])
                        nc.vector.tensor_sub(r[:, 0:cw], r[:, 0:cw], aps[0:N_OUT, 0:cw])
                        for ch in range(3):
                            nc.vector.tensor_mul(p[:, 0:cw], imgn_out[ch][:, c0:c1],
                                                 bps3[ch][0:N_OUT, 0:cw])
                            nc.vector.tensor_sub(r[:, 0:cw], r[:, 0:cw], p[:, 0:cw])
                        nc.vector.tensor_scalar(p[:, 0:cw], tf_o[:, c0:c1],
                                                float(c + 1), -TRI_CONF, OP.is_equal, OP.mult)
                        nc.vector.tensor_add(r[:, 0:cw], r[:, 0:cw], p[:, 0:cw])
                        sq = sc2.tile([N_OUT, CH0], F32, name="sq", tag="sq")
                        nc.scalar.activation(sq[:, 0:cw], r[:, 0:cw],
                                             mybir.ActivationFunctionType.Square,
                                             accum_out=acc_w[:, 2 * c + ci:2 * c + ci + 1])

                # ---- reduce this sub ----
                accv = sb.tile([N_OUT, 1], F32, name="accv", tag="accv")
                nc.vector.tensor_reduce(accv[:], acc_w[:, 0:14],
                                        axis=mybir.AxisListType.X, op=OP.add)
                if s == 0:
                    nc.vector.tensor_scalar(accm_tot[:], accv[:], rmask[:], None, OP.mult)
                else:
                    accm = sb.tile([N_OUT, 1], F32, name="accm", tag="accm")
                    nc.vector.tensor_scalar(accm[:], accv[:], rmask[:], None, OP.mult)
                    nc.vector.tensor_add(accm_tot[:], accm_tot[:], accm[:])

            fin_ps = psb.tile([1, 1], F32, name="fin", tag="bt")
            nc.tensor.matmul(fin_ps[:], accm_tot[:], ones[:], start=True, stop=True)
            fin = cst.tile([1, 1], F32, name="fin_sb", tag="fin_sb")
            nc.vector.tensor_copy(fin[:], fin_ps[:])
            nc.sync.dma_start(part_d[:], fin[:])

    nc.compile()
    return nc


def _get_program():
    global _PROGRAM
    if _PROGRAM is None:
        _PROGRAM = _build_program()
    return _PROGRAM


def _host_inputs(cprob, img_org, trimap):
    """Slice + pad full inputs into per-core input maps."""
    cprob = np.ascontiguousarray(cprob, dtype=np.float32)
    img_org = np.ascontiguousarray(img_org, dtype=np.float32)
    trimap = np.ascontiguousarray(trimap, dtype=np.int32)

    band = np.zeros((N_M, N_PIX), np.float32)
    for k in range(N_M):
        for m in range(N_PIX):
            if 0 <= k - m <= 2:
                band[k, m] = 1.0
    ones = np.ones((N_OUT, 1), np.float32)

    def rows(arr, lo, hi, fill):
        # arr[..., lo:hi, :] with zero/fill padding outside [0, H)
        lead = arr.shape[:-2]
        out = np.full(lead + (hi - lo, arr.shape[-1]), fill, arr.dtype)
        alo, ahi = max(lo, 0), min(hi, H)
        if ahi > alo:
            out[..., alo - lo:ahi - lo, :] = arr[..., alo:ahi, :]
        return out

    in_maps = []
    for core in range(8):
        b = core // 4
        subs = (2 * (core % 4), 2 * (core % 4) + 1)
        cpr = np.stack([rows(cprob[b], SUB_R0[s] - 2, SUB_R0[s] + N_PIX - 2, 0.0)
                        for s in subs])
        img = np.stack([rows(np.moveaxis(img_org[b], -1, 0), SUB_R0[s] - 2,
                             SUB_R0[s] + N_PIX - 2, 0.0) for s in subs])
        trm = np.stack([rows(trimap[b], SUB_R0[s] - 3, SUB_R0[s] + N_M - 3, 0)
                        for s in subs])
        vmask = np.zeros((2, N_WIN, 1), np.float32)
        rmask = np.zeros((2, N_OUT, 1), np.float32)
        for i, s in enumerate(subs):
            r0 = SUB_R0[s]
            for l in range(N_WIN):
                if 0 <= r0 - 2 + l < NWC:
                    vmask[i, l, 0] = 1.0 / 9.0
            own = 65 if s == 7 else 64
            rmask[i, 0:own, 0] = 1.0
        in_maps.append({
            "cpr": cpr, "img": img, "trim": trm,
            "band": band, "ones": ones,
            "vmask9": vmask, "rmask": rmask,
        })
    return in_maps


def run(cprob, img_org, trimap, trace=False):
    nc = _get_program()
    in_maps = _host_inputs(cprob, img_org, trimap)
    res = run_bass_kernel_spmd(nc, in_maps, list(range(8)), trace=trace)
    total = sum(float(r["partial"][0, 0]) for r in res.results)
    out = np.float32(total / (float(N) * float(N)))
    return out, res


def kernel(cprob, img_org, trimap):
    out, _ = run(cprob, img_org, trimap)
    return out



# revision 2
# speedup vs baseline: 1.0338x; 1.0338x over previous
"""Trainium2 Bass kernel v2 for nn_ClosedFormLoss (closed-form matting loss).

Layout: 8 cores = 2 images x 4 column-quarters.  Each core owns output pixel
cols [129q, 129q+129) of one image and ALL 513 rows, stored row-chunked:
partition p + chunk k <-> global row 128k+p.  All tiles use the full 128
partitions (vs 67-71 in v1), halving per-op free size.

Tiles (free dims):
  pixel level  [128, 5, 133]  cols [129q-2, 129q+131)
  window level [128, 4, 131]  cols [129q-2, 129q+129)  (rows 128k+m < 511)
  zero-padded window tiles [128, 6, 131] (chunks 1..4 = data) let the
  transpose-box row contraction be emitted as two uniform matmuls
  (main: dst chunk k <- src chunk k+1; boundary: dst chunk k <- src chunk k)
  trimap level [128, 7, 135]  (chunks 1..5 = data) same trick for dilate
  output level [128, 5, 129]  cols [129q, 129q+129)

Math per (image, class) with per-batch coefficients folded:
  S = box(o), q_ch = box(imgn_ch * o)       (PE, fp16 weights/movers)
  t_ch = q_ch - mu_ch * S                   (DVE fp16)
  bk_ch = sum_j invk[ch,j] * t_j            (invk = keep9 * inv(var), bf16)
  ak = keep9*S - sum_ch mu_ch * bk_ch
  A = boxT(ak), B_ch = boxT(bk_ch)          (PE bf16)
  r = o*Nkc - A - sum imgn_ch*B_ch - 100*(trimap==c+1)
  loss += sum r^2                            (DVE fused square+accum)
Row box filters are banded-matrix matmuls over partitions (+1 small boundary
matmul for the chunk seams); col filters are 3 shifted accumulations in the
same matmuls.  Moment boxes run in float32r (tf32) at full PE rate; det is
clamped at 1e-10 before reciprocal as insurance against tf32 rounding making
a near-singular det cross zero.
"""

import sys
import numpy as np
import ml_dtypes

sys.path.insert(0, "/opt/trn_rl_repo")

from concourse import bacc, mybir, tile  # noqa: E402
from concourse.bass_utils import run_bass_kernel_spmd  # noqa: E402

F32 = mybir.dt.float32
F32R = mybir.dt.float32r
F16 = mybir.dt.float16
BF16 = mybir.dt.bfloat16
OP = mybir.AluOpType
ACTF = mybir.ActivationFunctionType

N_CLASSES = 7
H = W = 513
NW = 511
N = H * W
OUT_C, WIN_C, PIX_C, T_C = 129, 131, 133, 135
KP, KW = 5, 4
EPS9 = 1e-7 / 9.0
TRI = 100.0
V9 = float(np.float32(np.float32(1.0 / 9.0).astype(ml_dtypes.bfloat16)))
NINE_EFF = float(1.0 / np.float32(V9))
DET_FLOOR = 1e-10

_PROGRAM = None


def _build_program():
    nc = bacc.Bacc("TRN2", target_bir_lowering=False, debug=False, num_devices=8)

    cpr_d = nc.declare_dram_parameter("cpr", [N_CLASSES, 128, KP, PIX_C], F16, isOutput=False)
    img_d = nc.declare_dram_parameter("img", [3, 128, KP, PIX_C], F16, isOutput=False)
    trim_d = nc.declare_dram_parameter("trim", [128, 7, T_C], F16, isOutput=False)
    vmask_d = nc.declare_dram_parameter("vmask", [128, KW, WIN_C], BF16, isOutput=False)
    w0f_d = nc.declare_dram_parameter("w0f", [128, 128], F16, isOutput=False)
    w0b_d = nc.declare_dram_parameter("w0b", [128, 128], BF16, isOutput=False)
    w1f_d = nc.declare_dram_parameter("w1f", [2, 128], F16, isOutput=False)
    w1b_d = nc.declare_dram_parameter("w1b", [2, 128], BF16, isOutput=False)
    wt0_d = nc.declare_dram_parameter("wt0", [128, 128], BF16, isOutput=False)
    wt1_d = nc.declare_dram_parameter("wt1", [128, 128], BF16, isOutput=False)
    wd0_d = nc.declare_dram_parameter("wd0", [128, 128], BF16, isOutput=False)
    wdu_d = nc.declare_dram_parameter("wdu", [128, 128], BF16, isOutput=False)
    wdd_d = nc.declare_dram_parameter("wdd", [128, 128], BF16, isOutput=False)
    ones_d = nc.declare_dram_parameter("ones", [128, 1], F32, isOutput=False)
    part_d = nc.declare_dram_parameter("partial", [1, 1], F32, isOutput=True)

    with tile.TileContext(nc) as tc:
        with (
            tc.tile_pool(name="cst", bufs=1) as cst,
            tc.tile_pool(name="pre", bufs=1) as pre,
            tc.tile_pool(name="spc", bufs=2) as spc,
            tc.tile_pool(name="sc2", bufs=2) as sc2,
            tc.tile_pool(name="akbk", bufs=2) as akbk,
            tc.tile_pool(name="psf", bufs=4, space="PSUM") as psf,
            tc.tile_pool(name="psb", bufs=4, space="PSUM") as psb,
        ):
            # ---- constant loads ----
            w0f = cst.tile([128, 128], F16, name="w0f", tag="w0f")
            w0b = cst.tile([128, 128], BF16, name="w0b", tag="w0b")
            w1f = cst.tile([2, 128], F16, name="w1f", tag="w1f")
            w1b = cst.tile([2, 128], BF16, name="w1b", tag="w1b")
            wt0 = cst.tile([128, 128], BF16, name="wt0", tag="wt0")
            wt1 = cst.tile([128, 128], BF16, name="wt1", tag="wt1")
            wd0 = cst.tile([128, 128], BF16, name="wd0", tag="wd0")
            wdu = cst.tile([128, 128], BF16, name="wdu", tag="wdu")
            wdd = cst.tile([128, 128], BF16, name="wdd", tag="wdd")
            ones = cst.tile([128, 1], F32, name="ones", tag="ones")
            vmask = cst.tile([128, KW, WIN_C], BF16, name="vmask", tag="vmask")
            trim32 = cst.tile([128, 7, T_C], F16, name="trim32", tag="trim32")
            imgf16 = [cst.tile([128, KP, PIX_C], F16, name=f"imgf16_{ch}", tag=f"imgf16_{ch}")
                      for ch in range(3)]
            for ch in range(3):
                nc.sync.dma_start(imgf16[ch][:], img_d[ch])
            for t, d in ((trim32, trim_d), (wd0, wd0_d), (wdu, wdu_d), (wdd, wdd_d),
                         (w0f, w0f_d), (w0b, w0b_d), (w1f, w1f_d),
                         (w1b, w1b_d), (wt0, wt0_d), (wt1, wt1_d), (ones, ones_d),
                         (vmask, vmask_d)):
                nc.gpsimd.dma_start(t[:], d[:])

            # ---- box helpers ----
            # psb pool tiles are uniform [128, 3, PIX_C] f32 (one PSUM bank);
            # each use slices the region it needs.
            def psb_pair():
                ta = psb.tile([128, 3, PIX_C], F32, name="tb_a", tag="tb")
                tb = psb.tile([128, 3, PIX_C], F32, name="tb_b", tag="tb")
                return ta, tb

            def fwd_box(ps_pair, src, wm, wb, colw=WIN_C):
                # window-level box: dst win chunk k <- pixel chunks k (main band,
                # row offsets 0..2) + k+1 (boundary rows 128..129)
                for ti, klo in enumerate((0, 2)):
                    ps = ps_pair[ti]
                    for dj in range(3):
                        nc.tensor.matmul(
                            ps[:, :, :], wm[:, :],
                            src[:, klo:klo + 2, dj:dj + colw],
                            start=(dj == 0), stop=False)
                        nc.tensor.matmul(
                            ps[:, :, :], wb[:, :],
                            src[0:2, klo + 1:klo + 3, dj:dj + colw],
                            start=False, stop=(dj == 2))

            def t_box(ps_a, ps_b, src6, colw=OUT_C):
                # pixel-level transpose box from zero-padded window tile
                pa = ps_a[:, 0:2, 0:colw]
                pb = ps_b[:, 0:3, 0:colw]
                for dd in range(3):
                    nc.tensor.matmul(pa, wt0[:, :], src6[:, 1:3, dd:dd + colw],
                                     start=(dd == 0), stop=False)
                    nc.tensor.matmul(ps_a[:, 1:2, 0:colw], wt1[:, :],
                                     src6[:, 1:2, dd:dd + colw],
                                     start=False, stop=(dd == 2))
                for dd in range(3):
                    nc.tensor.matmul(pb, wt0[:, :], src6[:, 3:6, dd:dd + colw],
                                     start=(dd == 0), stop=False)
                    nc.tensor.matmul(pb, wt1[:, :], src6[:, 2:5, dd:dd + colw],
                                     start=False, stop=(dd == 2))
                return pa, pb

            # ---- precompute: dilate -> d01 -> keep9 ----
            mdil = pre.tile([128, 7, T_C], BF16, name="mdil", tag="mdil")
            nc.vector.tensor_scalar(mdil[:], trim32[:], 128.0, None, OP.is_equal)
            dil_at, dil_bt = psb_pair()
            dil_a = dil_at[:, 0:2, 0:PIX_C]
            dil_b = dil_bt[:, 0:3, 0:PIX_C]
            for dd in range(3):
                nc.tensor.matmul(dil_a, wd0[:, :], mdil[:, 1:3, dd:dd + PIX_C],
                                 start=(dd == 0), stop=False)
                nc.tensor.matmul(dil_a, wdu[:, :], mdil[:, 2:4, dd:dd + PIX_C],
                                 start=False, stop=False)
                nc.tensor.matmul(dil_a, wdd[:, :], mdil[:, 0:2, dd:dd + PIX_C],
                                 start=False, stop=(dd == 2))
            for dd in range(3):
                nc.tensor.matmul(dil_b, wd0[:, :], mdil[:, 3:6, dd:dd + PIX_C],
                                 start=(dd == 0), stop=False)
                nc.tensor.matmul(dil_b, wdu[:, :], mdil[:, 4:7, dd:dd + PIX_C],
                                 start=False, stop=False)
                nc.tensor.matmul(dil_b, wdd[:, :], mdil[:, 2:5, dd:dd + PIX_C],
                                 start=False, stop=(dd == 2))
            d01 = pre.tile([128, KP, PIX_C], BF16, name="d01", tag="d01")
            nc.vector.tensor_scalar(d01[:, 0:2, :], dil_a, 0.0, None, OP.is_gt)
            nc.vector.tensor_scalar(d01[:, 2:5, :], dil_b, 0.0, None, OP.is_gt)

            k9_a = psf.tile([128, 2, WIN_C], F32, name="k9_a", tag="fwd")
            k9_b = psf.tile([128, 2, WIN_C], F32, name="k9_b", tag="fwd")
            fwd_box((k9_a, k9_b), d01, w0b, w1b)
            keep9 = cst.tile([128, 6, WIN_C], BF16, name="keep9", tag="keep9")
            nc.vector.memset(keep9[:, 0:1, :], 0.0)
            nc.vector.memset(keep9[:, 5:6, :], 0.0)
            nc.vector.tensor_scalar(keep9[:, 1:3, :], k9_a[:, :, :], 0.0, None, OP.is_gt)
            nc.vector.tensor_scalar(keep9[:, 3:5, :], k9_b[:, :, :], 0.0, None, OP.is_gt)
            nc.vector.tensor_tensor(keep9[:, 1:5, :], keep9[:, 1:5, :], vmask[:], OP.mult)

            # ---- Nkc = NINE_EFF * boxT(keep9) + conf ----
            nk_at, nk_bt = psb_pair()
            nk_a, nk_b = t_box(nk_at, nk_bt, keep9)
            nkc16 = cst.tile([128, KP, OUT_C], F16, name="nkc16", tag="nkc16")
            nc.vector.tensor_scalar(nkc16[:, 0:2, :], nk_a, NINE_EFF, TRI,
                                    OP.mult, OP.add)
            nc.vector.tensor_scalar(nkc16[:, 2:5, :], nk_b, NINE_EFF, TRI,
                                    OP.mult, OP.add)
            tf16 = cst.tile([128, KP, OUT_C], F16, name="tf16", tag="tf16")
            nc.vector.tensor_copy(tf16[:], trim32[:, 1:6, 3:3 + OUT_C])
            m100 = pre.tile([128, KP, OUT_C], F16, name="m100", tag="m100")
            nc.vector.tensor_scalar(m100[:], tf16[:], 128.0, TRI, OP.is_equal, OP.mult)
            nc.vector.tensor_tensor(nkc16[:], nkc16[:], m100[:], OP.subtract)

            # ---- per-class constants ----
            c100 = cst.tile([128, KP, OUT_C], F16, name="c100", tag="c100")
            nc.vector.memset(c100[:], -TRI)
            acc_tot = cst.tile([128, 1], F32, name="acc_tot", tag="acc_tot")
            nc.vector.memset(acc_tot[:], 0.0)

            # zero-pad chunks of the rotating ak/bk tiles (both buffers)
            for name in ("ak", "bk0", "bk1", "bk2"):
                for rep in range(2):
                    tt = akbk.tile([128, 6, WIN_C], BF16, name=f"{name}_z{rep}", tag=name)
                    nc.vector.memset(tt[:, 0:1, :], 0.0)
                    nc.vector.memset(tt[:, 5:6, :], 0.0)

            # ---- imgn fp16 (class pipeline) ----
            imgn16 = [cst.tile([128, KP, PIX_C], F16, name=f"imgn16_{ch}", tag=f"imgn16_{ch}")
                      for ch in range(3)]
            for ch in range(3):
                nc.scalar.activation(imgn16[ch][:], imgf16[ch][:], ACTF.Copy,
                                     bias=0.0, scale=1.0 / 255.0)

            # front(c): DMA + o16 + po + S/q boxes + psum->f16 copies.
            # No DVE ops, so fronts run ahead while DVE grinds the var chain
            # and the previous classes' matvec/residual.
            fronts = {}

            def front(c):
                o16 = spc.tile([128, KP, PIX_C], F16, name="o16", tag="o16", bufs=3)
                nc.sync.dma_start(o16[:], cpr_d[c])
                ps_s = (psf.tile([128, 2, WIN_C], F32, name="ps_s_a", tag="fwd"),
                        psf.tile([128, 2, WIN_C], F32, name="ps_s_b", tag="fwd"))
                fwd_box(ps_s, o16, w0f, w1f)
                s16 = sc2.tile([128, 1, KW, WIN_C], F16, name="s16", tag="s16", bufs=3)
                nc.scalar.activation(s16[:, 0, 0:2, :], ps_s[0][:, :, :], ACTF.Copy,
                                     bias=0.0, scale=1.0)
                nc.scalar.activation(s16[:, 0, 2:4, :], ps_s[1][:, :, :], ACTF.Copy,
                                     bias=0.0, scale=1.0)
                q_all = sc2.tile([128, 3, KW, WIN_C], F16, name="q_all",
                                 tag="q_all", bufs=3)
                for ch in range(3):
                    po = sc2.tile([128, KP, PIX_C], F16, name="po", tag="po")
                    nc.gpsimd.tensor_tensor(po[:], o16[:], imgn16[ch][:], OP.mult)
                    ps_q = (psf.tile([128, 2, WIN_C], F32, name="ps_q_a", tag="fwd"),
                            psf.tile([128, 2, WIN_C], F32, name="ps_q_b", tag="fwd"))
                    fwd_box(ps_q, po, w0f, w1f)
                    nc.scalar.activation(q_all[:, ch, 0:2, :], ps_q[0][:, :, :], ACTF.Copy,
                                         bias=0.0, scale=1.0)
                    nc.scalar.activation(q_all[:, ch, 2:4, :], ps_q[1][:, :, :], ACTF.Copy,
                                         bias=0.0, scale=1.0)
                fronts[c] = (o16, s16, q_all)

            # ---- color moments (float32r boxes on raw img, scales folded) ----
            pairs = [(0, 0), (0, 1), (0, 2), (1, 1), (1, 2), (2, 2)]
            mu_all = cst.tile([128, 3, KW, WIN_C], F16, name="mu_all", tag="mu_all")
            MUS = 1.0 / (9.0 * 255.0)
            for ch in range(3):
                mp_a = psf.tile([128, 2, WIN_C], F32, name="mp_a", tag="fwd")
                mp_b = psf.tile([128, 2, WIN_C], F32, name="mp_b", tag="fwd")
                fwd_box((mp_a, mp_b), imgf16[ch], w0f, w1f)
                nc.scalar.activation(mu_all[:, ch, 0:2, :], mp_a[:, :, :], ACTF.Copy,
                                     bias=0.0, scale=MUS)
                nc.scalar.activation(mu_all[:, ch, 2:4, :], mp_b[:, :, :], ACTF.Copy,
                                     bias=0.0, scale=MUS)
            E2S = 1.0 / (9.0 * 255.0 * 255.0)
            var = [pre.tile([128, KW, WIN_C], F16, name=f"var{i}", tag=f"var{i}")
                   for i in range(6)]
            eps_ab = []
            for i, (a, b) in enumerate(pairs):
                prod = pre.tile([128, KP, PIX_C], F16, name="prod", tag=f"prod{i % 3}")
                nc.vector.tensor_tensor(prod[:], imgf16[a][:], imgf16[b][:], OP.mult)
                e_a = psf.tile([128, 2, WIN_C], F32, name="e_a", tag="fwd")
                e_b = psf.tile([128, 2, WIN_C], F32, name="e_b", tag="fwd")
                fwd_box((e_a, e_b), prod, w0f, w1f)
                eps = EPS9 if a == b else 0.0
                e2 = pre.tile([128, KW, WIN_C], F16, name="e2", tag=f"e2_{i}")
                nc.scalar.activation(e2[:, 0:2, :], e_a[:, :, :], ACTF.Copy,
                                     bias=eps, scale=E2S)
                nc.scalar.activation(e2[:, 2:4, :], e_b[:, :, :], ACTF.Copy,
                                     bias=eps, scale=E2S)
                eps_ab.append(e2)
            for i, (a, b) in enumerate(pairs):
                mm = pre.tile([128, KW, WIN_C], F16, name="mm", tag=f"mm_{i % 2}")
                nc.vector.tensor_tensor(mm[:], mu16[a][:], mu16[b][:], OP.mult)
                nc.vector.tensor_tensor(var[i][:], eps_ab[i][:], mm[:], OP.subtract)
            v11, v12, v13, v22, v23, v33 = var

            front(0)
            front(1)

            # ---- adjugate / det / invk = keep9 * adj / max(det, floor) ----
            def fma_sub(x1, y1, x2, y2, tag):
                p1 = pre.tile([128, KW, WIN_C], F16, name="cof_p1", tag="cof_p1")
                p2 = pre.tile([128, KW, WIN_C], F16, name="cof_p2", tag="cof_p2")
                o = pre.tile([128, KW, WIN_C], F16, name=tag, tag=tag)
                nc.vector.tensor_tensor(p1[:], x1[:], y1[:], OP.mult)
                nc.vector.tensor_tensor(p2[:], x2[:], y2[:], OP.mult)
                nc.vector.tensor_tensor(o[:], p1[:], p2[:], OP.subtract)
                return o

            a11 = fma_sub(v22, v33, v23, v23, "a11")
            a12 = fma_sub(v13, v23, v12, v33, "a12")
            a13 = fma_sub(v12, v23, v13, v22, "a13")
            a22 = fma_sub(v11, v33, v13, v13, "a22")
            a23 = fma_sub(v12, v13, v11, v23, "a23")
            a33 = fma_sub(v11, v22, v12, v12, "a33")
            det = pre.tile([128, KW, WIN_C], F32, name="det", tag="det")
            dt2 = pre.tile([128, KW, WIN_C], F32, name="dt2", tag="dt2")
            nc.vector.tensor_tensor(det[:], v11[:], a11[:], OP.mult)
            nc.vector.tensor_tensor(dt2[:], v12[:], a12[:], OP.mult)
            nc.vector.tensor_tensor(det[:], det[:], dt2[:], OP.add)
            nc.vector.tensor_tensor(dt2[:], v13[:], a13[:], OP.mult)
            nc.vector.tensor_tensor(det[:], det[:], dt2[:], OP.add)
            nc.vector.tensor_scalar(det[:], det[:], DET_FLOOR, None, OP.max)
            rdet = pre.tile([128, KW, WIN_C], F32, name="rdet", tag="rdet")
            nc.vector.reciprocal(rdet[:], det[:])
            kr = pre.tile([128, KW, WIN_C], BF16, name="kr", tag="kr")
            nc.vector.tensor_tensor(kr[:], keep9[:, 1:5, :], rdet[:], OP.mult)
            invk = [cst.tile([128, KW, WIN_C], BF16, name=f"invk{i}", tag=f"invk{i}")
                    for i in range(6)]
            for i, adj in enumerate([a11, a12, a13, a22, a23, a33]):
                nc.vector.tensor_tensor(invk[i][:], kr[:], adj[:], OP.mult)
            ik = [[invk[0], invk[1], invk[2]],
                  [invk[1], invk[3], invk[4]],
                  [invk[2], invk[4], invk[5]]]

            sqd_tiles = {}

            def sqd_dummy(c):
                t = sc2.tile([128, KP, OUT_C], F16, name="sqd", tag="sqd")
                sqd_tiles[c] = t
                return t[:]

            # ---- class loop ----
            for c in range(N_CLASSES):
                o16, s16, q_all = fronts.pop(c)

                # t_ch = q_ch - mu_ch*S ; bk_ch = sum_j ikrow[j][ch]*t_j ;
                # ak = keep9*S - sum mu_ch*bk_ch   (per-channel ops for overlap)
                tch = []
                for ch in range(3):
                    ms = sc2.tile([128, KW, WIN_C], F16, name="ms", tag="ms")
                    nc.vector.tensor_tensor(ms[:], mu_all[:, ch], s16[:, 0], OP.mult)
                    tt = sc2.tile([128, KW, WIN_C], F16, name=f"t_{ch}", tag=f"t_{ch}")
                    nc.vector.tensor_tensor(tt[:], q_all[:, ch], ms[:], OP.subtract)
                    tch.append(tt)

                ab16 = [None] * 4

                def emit_tbox(fi, src6):
                    pat, pbt = psb_pair()
                    pa, pb = t_box(pat, pbt, src6)
                    f16t = sc2.tile([128, KP, OUT_C], F16, name=f"ab16_{fi}", tag=f"ab16_{fi}")
                    nc.scalar.activation(f16t[:, 0:2, :], pa, ACTF.Copy,
                                         bias=0.0, scale=1.0)
                    nc.scalar.activation(f16t[:, 2:5, :], pb, ACTF.Copy,
                                         bias=0.0, scale=1.0)
                    ab16[fi] = f16t

                bks = []
                for ch in range(3):
                    bk = akbk.tile([128, 6, WIN_C], BF16, name=f"bk{ch}", tag=f"bk{ch}")
                    bkd = bk[:, 1:5, :]
                    nc.vector.tensor_tensor(bkd, ikrow[0][:, ch], tch[0][:], OP.mult)
                    bp = sc2.tile([128, KW, WIN_C], BF16, name="bp", tag="bp")
                    nc.vector.tensor_tensor(bp[:], ikrow[1][:, ch], tch[1][:], OP.mult)
                    nc.vector.tensor_tensor(bkd, bkd, bp[:], OP.add)
                    nc.vector.tensor_tensor(bp[:], ikrow[2][:, ch], tch[2][:], OP.mult)
                    nc.vector.tensor_tensor(bkd, bkd, bp[:], OP.add)
                    bks.append(bk)
                    emit_tbox(1 + ch, bk)

                # ak = keep9*S - sum mu_ch * bk_ch
                ak = akbk.tile([128, 6, WIN_C], BF16, name="ak", tag="ak")
                akd = ak[:, 1:5, :]
                nc.vector.tensor_tensor(akd, keep9[:, 1:5, :], s16[:, 0], OP.mult)
                for ch in range(3):
                    am = sc2.tile([128, KW, WIN_C], BF16, name="am", tag="am")
                    nc.vector.tensor_tensor(am[:], mu_all[:, ch], bks[ch][:, 1:5, :], OP.mult)
                    nc.vector.tensor_tensor(akd, akd, am[:], OP.subtract)
                emit_tbox(0, ak)

                # residual: B-terms first (their copies land earliest), A last
                rps = []
                for ch in range(3):
                    rpp = sc2.tile([128, KP, OUT_C], F16, name=f"rpp_{ch}", tag=f"rpp_{ch}")
                    nc.gpsimd.tensor_tensor(rpp[:], imgn16[ch][:, :, 2:2 + OUT_C],
                                            ab16[1 + ch][:], OP.mult)
                    rps.append(rpp)
                r = sc2.tile([128, KP, OUT_C], F16, name="r", tag="r")
                nc.vector.tensor_tensor(r[:], o16[:, :, 2:2 + OUT_C], nkc16[:], OP.mult)
                rp = sc2.tile([128, KP, OUT_C], F16, name="rp", tag="rp")
                nc.vector.scalar_tensor_tensor(rp[:], tf16[:], float(c + 1), c100[:],
                                               OP.is_equal, OP.mult)
                nc.vector.tensor_tensor(r[:], r[:], rp[:], OP.add)
                for ch in range(3):
                    nc.vector.tensor_tensor(r[:], r[:], rps[ch][:], OP.subtract)
                nc.vector.tensor_tensor(r[:], r[:], ab16[0][:], OP.subtract)
                acc_c = sc2.tile([128, 1], F32, name="acc_c", tag="acc_c")
                nc.scalar.activation(sqd_dummy(c), r[:], ACTF.Square,
                                     bias=0.0, scale=1.0, accum_out=acc_c[:])
                nc.vector.tensor_tensor(acc_tot[:], acc_tot[:], acc_c[:], OP.add)

                if c + 2 < N_CLASSES:
                    front(c + 2)

            # ---- final reduce over partitions ----
            fin_ps = psb.tile([128, 3, PIX_C], F32, name="fin", tag="tb")
            nc.tensor.matmul(fin_ps[0:1, 0:1, 0:1], acc_tot[:], ones[:],
                             start=True, stop=True)
            fin = cst.tile([1, 1], F32, name="fin_sb", tag="fin_sb")
            nc.vector.tensor_copy(fin[:], fin_ps[0:1, 0:1, 0:1])
            nc.sync.dma_start(part_d[:], fin[:])

    nc.compile()
    return nc


def _get_program():
    global _PROGRAM
    if _PROGRAM is None:
        _PROGRAM = _build_program()
    return _PROGRAM


# ---------------- host-side prep ----------------

def _chunk_rows(arr, n_chunks, row_base=0):
    """arr [..., R, C] global rows -> [..., 128, n_chunks, C] with row
    128k+p placed at (p, k); row_base shifts arr's row 0 to chunk row_base*128."""
    lead = arr.shape[:-2]
    rr, cc = arr.shape[-2], arr.shape[-1]
    out = np.zeros(lead + (n_chunks * 128, cc), arr.dtype)
    out[..., row_base * 128:row_base * 128 + rr, :] = arr
    out = out.reshape(lead + (n_chunks, 128, cc))
    return np.moveaxis(out, -3, -2)  # [..., 128, n_chunks, cc]


def _col_slice(arr, c0, width):
    lead = arr.shape[:-1]
    out = np.zeros(lead + (width,), arr.dtype)
    alo, ahi = max(c0, 0), min(c0 + width, arr.shape[-1])
    if ahi > alo:
        out[..., alo - c0:ahi - c0] = arr[..., alo:ahi]
    return out


def _band(n_src, n_dst, offs, rows=None):
    w = np.zeros((n_src, n_dst), np.float32)
    src_rows = range(n_src) if rows is None else rows
    for si, sr in enumerate(src_rows):
        for m in range(n_dst):
            if sr - m in offs:
                w[si, m] = 1.0
    return w


def _host_inputs(cprob, img_org, trimap):
    cprob = np.ascontiguousarray(cprob, dtype=np.float32)
    img_org = np.ascontiguousarray(img_org, dtype=np.float32)
    trimap_f = np.ascontiguousarray(trimap, dtype=np.float32)

    w0 = _band(128, 128, (0, 1, 2))
    w1 = _band(2, 128, (0, 1, 2), rows=[128, 129])
    wt0 = _band(128, 128, (0, -1, -2))          # wt0[m,p]=1 if 0<=p-m<=2
    wt1 = np.zeros((128, 128), np.float32)
    wt1[126, 0] = wt1[127, 0] = wt1[127, 1] = 1.0
    wd0 = _band(128, 128, (-1, 0, 1))
    wdu = np.zeros((128, 128), np.float32)
    wdu[0, 127] = 1.0
    wdd = np.zeros((128, 128), np.float32)
    wdd[127, 0] = 1.0
    ones = np.ones((128, 1), np.float32)
    bf = ml_dtypes.bfloat16
    v9 = np.float32(np.float32(1.0 / 9.0).astype(bf))

    in_maps = []
    for core in range(8):
        b, q = core // 4, core % 4
        cb = 129 * q
        cpr = _chunk_rows(_col_slice(cprob[b], cb - 2, PIX_C), KP).astype(np.float16)
        img = _chunk_rows(_col_slice(np.moveaxis(img_org[b], -1, 0), cb - 2, PIX_C), KP).astype(np.float16)
        trm = _chunk_rows(_col_slice(trimap_f[b], cb - 3, T_C), 7, row_base=1).astype(np.float16)
        vm = np.zeros((128, KW, WIN_C), np.float32)
        rows_valid = (np.arange(128)[:, None] + 128 * np.arange(KW)[None, :]) < NW
        cols_valid = (cb - 2 + np.arange(WIN_C) >= 0) & (cb - 2 + np.arange(WIN_C) < NW)
        vm[rows_valid[:, :, None] & cols_valid[None, None, :]] = v9
        in_maps.append({
            "cpr": cpr, "img": img, "trim": trm,
            "vmask": vm.astype(bf),
            "w0f": w0.astype(np.float16), "w0b": w0.astype(bf),
            "w1f": w1.astype(np.float16), "w1b": w1.astype(bf),
            "wt0": wt0.astype(bf), "wt1": wt1.astype(bf),
            "wd0": wd0.astype(bf), "wdu": wdu.astype(bf), "wdd": wdd.astype(bf),
            "ones": ones,
        })
    return in_maps


def run(cprob, img_org, trimap, trace=False):
    nc = _get_program()
    in_maps = _host_inputs(cprob, img_org, trimap)
    res = run_bass_kernel_spmd(nc, in_maps, list(range(8)), trace=trace)
    total = sum(float(r["partial"][0, 0]) for r in res.results)
    out = np.float32(total / (float(N) * float(N)))
    return out, res


def kernel(cprob, img_org, trimap):
    out, _ = run(cprob, img_org, trimap)
    return out


# revision 3
# speedup vs baseline: 1.0423x; 1.0082x over previous
"""Trainium2 Bass kernel v2 for nn_ClosedFormLoss (closed-form matting loss).

Layout: 8 cores = 2 images x 4 column-quarters.  Each core owns output pixel
cols [129q, 129q+129) of one image and ALL 513 rows, stored row-chunked:
partition p + chunk k <-> global row 128k+p.  All tiles use the full 128
partitions (vs 67-71 in v1), halving per-op free size.

Tiles (free dims):
  pixel level  [128, 5, 133]  cols [129q-2, 129q+131)
  window level [128, 4, 131]  cols [129q-2, 129q+129)  (rows 128k+m < 511)
  zero-padded window tiles [128, 6, 131] (chunks 1..4 = data) let the
  transpose-box row contraction be emitted as two uniform matmuls
  (main: dst chunk k <- src chunk k+1; boundary: dst chunk k <- src chunk k)
  trimap level [128, 7, 135]  (chunks 1..5 = data) same trick for dilate
  output level [128, 5, 129]  cols [129q, 129q+129)

Math per (image, class) with per-batch coefficients folded:
  S = box(o), q_ch = box(imgn_ch * o)       (PE, fp16 weights/movers)
  t_ch = q_ch - mu_ch * S                   (DVE fp16)
  bk_ch = sum_j invk[ch,j] * t_j            (invk = keep9 * inv(var), bf16)
  ak = keep9*S - sum_ch mu_ch * bk_ch
  A = boxT(ak), B_ch = boxT(bk_ch)          (PE bf16)
  r = o*Nkc - A - sum imgn_ch*B_ch - 100*(trimap==c+1)
  loss += sum r^2                            (DVE fused square+accum)
Row box filters are banded-matrix matmuls over partitions (+1 small boundary
matmul for the chunk seams); col filters are 3 shifted accumulations in the
same matmuls.  Moment boxes run in float32r (tf32) at full PE rate; det is
clamped at 1e-10 before reciprocal as insurance against tf32 rounding making
a near-singular det cross zero.
"""

import sys
import numpy as np
import ml_dtypes

sys.path.insert(0, "/opt/trn_rl_repo")

from concourse import bacc, mybir, tile  # noqa: E402
from concourse.bass_utils import run_bass_kernel_spmd  # noqa: E402

F32 = mybir.dt.float32
F32R = mybir.dt.float32r
F16 = mybir.dt.float16
BF16 = mybir.dt.bfloat16
OP = mybir.AluOpType
ACTF = mybir.ActivationFunctionType

N_CLASSES = 7
H = W = 513
NW = 511
N = H * W
OUT_C, WIN_C, PIX_C, T_C = 129, 131, 133, 135
KP, KW = 5, 4
EPS9 = 1e-7 / 9.0
TRI = 100.0
V9 = float(np.float32(np.float32(1.0 / 9.0).astype(ml_dtypes.bfloat16)))
NINE_EFF = float(1.0 / np.float32(V9))
DET_FLOOR = 1e-10

_PROGRAM = None


def _build_program():
    nc = bacc.Bacc("TRN2", target_bir_lowering=False, debug=False, num_devices=8)

    cpr_d = nc.declare_dram_parameter("cpr", [N_CLASSES, 128, KP, PIX_C], F16, isOutput=False)
    img_d = nc.declare_dram_parameter("img", [3, 128, KP, PIX_C], F16, isOutput=False)
    trim_d = nc.declare_dram_parameter("trim", [128, 7, T_C], F16, isOutput=False)
    vmask_d = nc.declare_dram_parameter("vmask", [128, KW, WIN_C], BF16, isOutput=False)
    w0f_d = nc.declare_dram_parameter("w0f", [128, 128], F16, isOutput=False)
    w0b_d = nc.declare_dram_parameter("w0b", [128, 128], BF16, isOutput=False)
    w1f_d = nc.declare_dram_parameter("w1f", [2, 128], F16, isOutput=False)
    w1b_d = nc.declare_dram_parameter("w1b", [2, 128], BF16, isOutput=False)
    wt0_d = nc.declare_dram_parameter("wt0", [128, 128], BF16, isOutput=False)
    wt1_d = nc.declare_dram_parameter("wt1", [128, 128], BF16, isOutput=False)
    wd0_d = nc.declare_dram_parameter("wd0", [128, 128], BF16, isOutput=False)
    wdu_d = nc.declare_dram_parameter("wdu", [128, 128], BF16, isOutput=False)
    wdd_d = nc.declare_dram_parameter("wdd", [128, 128], BF16, isOutput=False)
    ones_d = nc.declare_dram_parameter("ones", [128, 1], F32, isOutput=False)
    part_d = nc.declare_dram_parameter("partial", [1, 1], F32, isOutput=True)

    with tile.TileContext(nc) as tc:
        with (
            tc.tile_pool(name="cst", bufs=1) as cst,
            tc.tile_pool(name="pre", bufs=1) as pre,
            tc.tile_pool(name="spc", bufs=2) as spc,
            tc.tile_pool(name="sc2", bufs=2) as sc2,
            tc.tile_pool(name="akbk", bufs=2) as akbk,
            tc.tile_pool(name="psf", bufs=4, space="PSUM") as psf,
            tc.tile_pool(name="psb", bufs=4, space="PSUM") as psb,
        ):
            # ---- constant loads ----
            w0f = cst.tile([128, 128], F16, name="w0f", tag="w0f")
            w0b = cst.tile([128, 128], BF16, name="w0b", tag="w0b")
            w1f = cst.tile([2, 128], F16, name="w1f", tag="w1f")
            w1b = cst.tile([2, 128], BF16, name="w1b", tag="w1b")
            wt0 = cst.tile([128, 128], BF16, name="wt0", tag="wt0")
            wt1 = cst.tile([128, 128], BF16, name="wt1", tag="wt1")
            wd0 = cst.tile([128, 128], BF16, name="wd0", tag="wd0")
            wdu = cst.tile([128, 128], BF16, name="wdu", tag="wdu")
            wdd = cst.tile([128, 128], BF16, name="wdd", tag="wdd")
            ones = cst.tile([128, 1], F32, name="ones", tag="ones")
            vmask = cst.tile([128, KW, WIN_C], BF16, name="vmask", tag="vmask")
            trim32 = cst.tile([128, 7, T_C], F16, name="trim32", tag="trim32")
            imgf16 = [cst.tile([128, KP, PIX_C], F16, name=f"imgf16_{ch}", tag=f"imgf16_{ch}")
                      for ch in range(3)]
            for ch in range(3):
                nc.sync.dma_start(imgf16[ch][:], img_d[ch])
            for t, d in ((trim32, trim_d), (wd0, wd0_d), (wdu, wdu_d), (wdd, wdd_d),
                         (w0f, w0f_d), (w0b, w0b_d), (w1f, w1f_d),
                         (w1b, w1b_d), (wt0, wt0_d), (wt1, wt1_d), (ones, ones_d),
                         (vmask, vmask_d)):
                nc.gpsimd.dma_start(t[:], d[:])

            # ---- box helpers ----
            # psb pool tiles are uniform [128, 3, PIX_C] f32 (one PSUM bank);
            # each use slices the region it needs.
            def psb_pair():
                ta = psb.tile([128, 3, PIX_C], F32, name="tb_a", tag="tb")
                tb = psb.tile([128, 3, PIX_C], F32, name="tb_b", tag="tb")
                return ta, tb

            def fwd_box(ps_pair, src, wm, wb, colw=WIN_C):
                # window-level box: dst win chunk k <- pixel chunks k (main band,
                # row offsets 0..2) + k+1 (boundary rows 128..129)
                for ti, klo in enumerate((0, 2)):
                    ps = ps_pair[ti]
                    for dj in range(3):
                        nc.tensor.matmul(
                            ps[:, :, :], wm[:, :],
                            src[:, klo:klo + 2, dj:dj + colw],
                            start=(dj == 0), stop=False)
                        nc.tensor.matmul(
                            ps[:, :, :], wb[:, :],
                            src[0:2, klo + 1:klo + 3, dj:dj + colw],
                            start=False, stop=(dj == 2))

            def t_box(ps_a, ps_b, src6, colw=OUT_C):
                # pixel-level transpose box from zero-padded window tile
                pa = ps_a[:, 0:2, 0:colw]
                pb = ps_b[:, 0:3, 0:colw]
                for dd in range(3):
                    nc.tensor.matmul(pa, wt0[:, :], src6[:, 1:3, dd:dd + colw],
                                     start=(dd == 0), stop=False)
                    nc.tensor.matmul(ps_a[:, 1:2, 0:colw], wt1[:, :],
                                     src6[:, 1:2, dd:dd + colw],
                                     start=False, stop=(dd == 2))
                for dd in range(3):
                    nc.tensor.matmul(pb, wt0[:, :], src6[:, 3:6, dd:dd + colw],
                                     start=(dd == 0), stop=False)
                    nc.tensor.matmul(pb, wt1[:, :], src6[:, 2:5, dd:dd + colw],
                                     start=False, stop=(dd == 2))
                return pa, pb

            # ---- precompute: dilate -> d01 -> keep9 ----
            mdil = pre.tile([128, 7, T_C], BF16, name="mdil", tag="mdil")
            nc.vector.tensor_scalar(mdil[:], trim32[:], 128.0, None, OP.is_equal)
            dil_at, dil_bt = psb_pair()
            dil_a = dil_at[:, 0:2, 0:PIX_C]
            dil_b = dil_bt[:, 0:3, 0:PIX_C]
            for dd in range(3):
                nc.tensor.matmul(dil_a, wd0[:, :], mdil[:, 1:3, dd:dd + PIX_C],
                                 start=(dd == 0), stop=False)
                nc.tensor.matmul(dil_a, wdu[:, :], mdil[:, 2:4, dd:dd + PIX_C],
                                 start=False, stop=False)
                nc.tensor.matmul(dil_a, wdd[:, :], mdil[:, 0:2, dd:dd + PIX_C],
                                 start=False, stop=(dd == 2))
            for dd in range(3):
                nc.tensor.matmul(dil_b, wd0[:, :], mdil[:, 3:6, dd:dd + PIX_C],
                                 start=(dd == 0), stop=False)
                nc.tensor.matmul(dil_b, wdu[:, :], mdil[:, 4:7, dd:dd + PIX_C],
                                 start=False, stop=False)
                nc.tensor.matmul(dil_b, wdd[:, :], mdil[:, 2:5, dd:dd + PIX_C],
                                 start=False, stop=(dd == 2))
            d01 = pre.tile([128, KP, PIX_C], BF16, name="d01", tag="d01")
            nc.vector.tensor_scalar(d01[:, 0:2, :], dil_a, 0.0, None, OP.is_gt)
            nc.vector.tensor_scalar(d01[:, 2:5, :], dil_b, 0.0, None, OP.is_gt)

            k9_a = psf.tile([128, 2, WIN_C], F32, name="k9_a", tag="fwd")
            k9_b = psf.tile([128, 2, WIN_C], F32, name="k9_b", tag="fwd")
            fwd_box((k9_a, k9_b), d01, w0b, w1b)
            keep9 = cst.tile([128, 6, WIN_C], BF16, name="keep9", tag="keep9")
            nc.vector.memset(keep9[:, 0:1, :], 0.0)
            nc.vector.memset(keep9[:, 5:6, :], 0.0)
            nc.vector.tensor_scalar(keep9[:, 1:3, :], k9_a[:, :, :], 0.0, None, OP.is_gt)
            nc.vector.tensor_scalar(keep9[:, 3:5, :], k9_b[:, :, :], 0.0, None, OP.is_gt)
            nc.vector.tensor_tensor(keep9[:, 1:5, :], keep9[:, 1:5, :], vmask[:], OP.mult)

            # ---- Nkc = NINE_EFF * boxT(keep9) + conf ----
            nk_at, nk_bt = psb_pair()
            nk_a, nk_b = t_box(nk_at, nk_bt, keep9)
            nkc16 = cst.tile([128, KP, OUT_C], F16, name="nkc16", tag="nkc16")
            nc.scalar.activation(nkc16[:, 0:2, :], nk_a, ACTF.Copy,
                                 bias=TRI, scale=NINE_EFF)
            nc.scalar.activation(nkc16[:, 2:5, :], nk_b, ACTF.Copy,
                                 bias=TRI, scale=NINE_EFF)
            tf16 = cst.tile([128, KP, OUT_C], F16, name="tf16", tag="tf16")
            nc.vector.tensor_copy(tf16[:], trim32[:, 1:6, 3:3 + OUT_C])
            m100 = pre.tile([128, KP, OUT_C], F16, name="m100", tag="m100")
            nc.vector.tensor_scalar(m100[:], tf16[:], 128.0, TRI, OP.is_equal, OP.mult)
            nc.vector.tensor_tensor(nkc16[:], nkc16[:], m100[:], OP.subtract)

            # ---- per-class constants ----
            c100 = cst.tile([128, KP, OUT_C], F16, name="c100", tag="c100")
            nc.vector.memset(c100[:], -TRI)
            acc_tot = cst.tile([128, 1], F32, name="acc_tot", tag="acc_tot")
            nc.vector.memset(acc_tot[:], 0.0)

            # zero-pad chunks of the rotating ak/bk tiles (both buffers)
            for name in ("ak", "bk0", "bk1", "bk2"):
                for rep in range(2):
                    tt = akbk.tile([128, 6, WIN_C], BF16, name=f"{name}_z{rep}", tag=name)
                    nc.vector.memset(tt[:, 0:1, :], 0.0)
                    nc.vector.memset(tt[:, 5:6, :], 0.0)

            # ---- imgn fp16 (class pipeline) ----
            imgn16 = [cst.tile([128, KP, PIX_C], F16, name=f"imgn16_{ch}", tag=f"imgn16_{ch}")
                      for ch in range(3)]
            for ch in range(3):
                nc.scalar.activation(imgn16[ch][:], imgf16[ch][:], ACTF.Copy,
                                     bias=0.0, scale=1.0 / 255.0)

            # front(c): DMA + o16 + po + S/q boxes + psum->f16 copies.
            # No DVE ops, so fronts run ahead while DVE grinds the var chain
            # and the previous classes' matvec/residual.
            fronts = {}

            def front(c):
                o16 = spc.tile([128, KP, PIX_C], F16, name="o16", tag="o16", bufs=3)
                nc.sync.dma_start(o16[:], cpr_d[c])
                ps_s = (psf.tile([128, 2, WIN_C], F32, name="ps_s_a", tag="fwd"),
                        psf.tile([128, 2, WIN_C], F32, name="ps_s_b", tag="fwd"))
                fwd_box(ps_s, o16, w0f, w1f)
                s16 = sc2.tile([128, 1, KW, WIN_C], F16, name="s16", tag="s16", bufs=3)
                nc.scalar.activation(s16[:, 0, 0:2, :], ps_s[0][:, :, :], ACTF.Copy,
                                     bias=0.0, scale=1.0)
                nc.scalar.activation(s16[:, 0, 2:4, :], ps_s[1][:, :, :], ACTF.Copy,
                                     bias=0.0, scale=1.0)
                q_all = sc2.tile([128, 3, KW, WIN_C], F16, name="q_all",
                                 tag="q_all", bufs=3)
                for ch in range(3):
                    po = sc2.tile([128, KP, PIX_C], F16, name="po", tag="po")
                    nc.gpsimd.tensor_tensor(po[:], o16[:], imgn16[ch][:], OP.mult)
                    ps_q = (psf.tile([128, 2, WIN_C], F32, name="ps_q_a", tag="fwd"),
                            psf.tile([128, 2, WIN_C], F32, name="ps_q_b", tag="fwd"))
                    fwd_box(ps_q, po, w0f, w1f)
                    nc.scalar.activation(q_all[:, ch, 0:2, :], ps_q[0][:, :, :], ACTF.Copy,
                                         bias=0.0, scale=1.0)
                    nc.scalar.activation(q_all[:, ch, 2:4, :], ps_q[1][:, :, :], ACTF.Copy,
                                         bias=0.0, scale=1.0)
                fronts[c] = (o16, s16, q_all)

            # ---- color moments (float32r boxes on raw img, scales folded) ----
            pairs = [(0, 0), (0, 1), (0, 2), (1, 1), (1, 2), (2, 2)]
            mu_all = cst.tile([128, 3, KW, WIN_C], F16, name="mu_all", tag="mu_all")
            MUS = 1.0 / (9.0 * 255.0)
            for ch in range(3):
                mp_a = psf.tile([128, 2, WIN_C], F32, name="mp_a", tag="fwd")
                mp_b = psf.tile([128, 2, WIN_C], F32, name="mp_b", tag="fwd")
                fwd_box((mp_a, mp_b), imgf16[ch], w0f, w1f)
                nc.scalar.activation(mu_all[:, ch, 0:2, :], mp_a[:, :, :], ACTF.Copy,
                                     bias=0.0, scale=MUS)
                nc.scalar.activation(mu_all[:, ch, 2:4, :], mp_b[:, :, :], ACTF.Copy,
                                     bias=0.0, scale=MUS)
            E2S = 1.0 / (9.0 * 255.0 * 255.0)
            var = [pre.tile([128, KW, WIN_C], F16, name=f"var{i}", tag=f"var{i}")
                   for i in range(6)]
            eps_ab = []
            for i, (a, b) in enumerate(pairs):
                prod = pre.tile([128, KP, PIX_C], F16, name="prod", tag=f"prod{i % 3}")
                nc.vector.tensor_tensor(prod[:], imgf16[a][:], imgf16[b][:], OP.mult)
                e_a = psf.tile([128, 2, WIN_C], F32, name="e_a", tag="fwd")
                e_b = psf.tile([128, 2, WIN_C], F32, name="e_b", tag="fwd")
                fwd_box((e_a, e_b), prod, w0f, w1f)
                eps = EPS9 if a == b else 0.0
                e2 = pre.tile([128, KW, WIN_C], F16, name="e2", tag=f"e2_{i}")
                nc.scalar.activation(e2[:, 0:2, :], e_a[:, :, :], ACTF.Copy,
                                     bias=eps, scale=E2S)
                nc.scalar.activation(e2[:, 2:4, :], e_b[:, :, :], ACTF.Copy,
                                     bias=eps, scale=E2S)
                eps_ab.append(e2)
            for i, (a, b) in enumerate(pairs):
                mm = pre.tile([128, KW, WIN_C], F16, name="mm", tag=f"mm_{i % 2}")
                nc.vector.tensor_tensor(mm[:], mu16[a][:], mu16[b][:], OP.mult)
                nc.vector.tensor_tensor(var[i][:], eps_ab[i][:], mm[:], OP.subtract)
            v11, v12, v13, v22, v23, v33 = var

            front(0)
            front(1)

            # ---- adjugate / det / invk = keep9 * adj / max(det, floor) ----
            def fma_sub(x1, y1, x2, y2, tag, eng=None):
                e = eng or nc.vector
                p1 = pre.tile([128, KW, WIN_C], F16, name="cof_p1", tag=f"cof_p1{tag}")
                p2 = pre.tile([128, KW, WIN_C], F16, name="cof_p2", tag=f"cof_p2{tag}")
                o = pre.tile([128, KW, WIN_C], F16, name=tag, tag=tag)
                e.tensor_tensor(p1[:], x1[:], y1[:], OP.mult)
                e.tensor_tensor(p2[:], x2[:], y2[:], OP.mult)
                e.tensor_tensor(o[:], p1[:], p2[:], OP.subtract)
                return o

            a11 = fma_sub(v22, v33, v23, v23, "a11")
            a12 = fma_sub(v13, v23, v12, v33, "a12")
            a13 = fma_sub(v12, v23, v13, v22, "a13")
            # det does not need a22/a23/a33 -- their Pool latency hides
            # behind the det/rdet tail of the chain
            a22 = fma_sub(v11, v33, v13, v13, "a22", nc.gpsimd)
            a23 = fma_sub(v12, v13, v11, v23, "a23", nc.gpsimd)
            a33 = fma_sub(v11, v22, v12, v12, "a33", nc.gpsimd)
            det = pre.tile([128, KW, WIN_C], F32, name="det", tag="det")
            dt2 = pre.tile([128, KW, WIN_C], F32, name="dt2", tag="dt2")
            nc.vector.tensor_tensor(det[:], v11[:], a11[:], OP.mult)
            nc.vector.tensor_tensor(dt2[:], v12[:], a12[:], OP.mult)
            nc.vector.tensor_tensor(det[:], det[:], dt2[:], OP.add)
            nc.vector.tensor_tensor(dt2[:], v13[:], a13[:], OP.mult)
            nc.vector.tensor_tensor(det[:], det[:], dt2[:], OP.add)
            nc.vector.tensor_scalar(det[:], det[:], DET_FLOOR, None, OP.max)
            rdet = pre.tile([128, KW, WIN_C], F32, name="rdet", tag="rdet")
            nc.vector.reciprocal(rdet[:], det[:])
            kr = pre.tile([128, KW, WIN_C], BF16, name="kr", tag="kr")
            nc.vector.tensor_tensor(kr[:], keep9[:, 1:5, :], rdet[:], OP.mult)
            invk = [cst.tile([128, KW, WIN_C], BF16, name=f"invk{i}", tag=f"invk{i}")
                    for i in range(6)]
            for i, adj in enumerate([a11, a12, a13, a22, a23, a33]):
                nc.vector.tensor_tensor(invk[i][:], kr[:], adj[:], OP.mult)
            ik = [[invk[0], invk[1], invk[2]],
                  [invk[1], invk[3], invk[4]],
                  [invk[2], invk[4], invk[5]]]

            sqd_tiles = {}

            def sqd_dummy(c):
                t = sc2.tile([128, KP, OUT_C], F16, name="sqd", tag="sqd")
                sqd_tiles[c] = t
                return t[:]

            # ---- class loop ----
            for c in range(N_CLASSES):
                o16, s16, q_all = fronts.pop(c)

                # t_ch = q_ch - mu_ch*S ; bk_ch = sum_j ikrow[j][ch]*t_j ;
                # ak = keep9*S - sum mu_ch*bk_ch   (per-channel ops for overlap)
                tch = []
                for ch in range(3):
                    ms = sc2.tile([128, KW, WIN_C], F16, name="ms", tag="ms")
                    nc.vector.tensor_tensor(ms[:], mu_all[:, ch], s16[:, 0], OP.mult)
                    tt = sc2.tile([128, KW, WIN_C], F16, name=f"t_{ch}", tag=f"t_{ch}")
                    nc.vector.tensor_tensor(tt[:], q_all[:, ch], ms[:], OP.subtract)
                    tch.append(tt)

                ab16 = [None] * 4

                def emit_tbox(fi, src6):
                    pat, pbt = psb_pair()
                    pa, pb = t_box(pat, pbt, src6)
                    f16t = sc2.tile([128, KP, OUT_C], F16, name=f"ab16_{fi}", tag=f"ab16_{fi}")
                    nc.scalar.activation(f16t[:, 0:2, :], pa, ACTF.Copy,
                                         bias=0.0, scale=1.0)
                    nc.scalar.activation(f16t[:, 2:5, :], pb, ACTF.Copy,
                                         bias=0.0, scale=1.0)
                    ab16[fi] = f16t

                bks = []
                for ch in range(3):
                    bk = akbk.tile([128, 6, WIN_C], BF16, name=f"bk{ch}", tag=f"bk{ch}")
                    bkd = bk[:, 1:5, :]
                    nc.vector.tensor_tensor(bkd, ikrow[0][:, ch], tch[0][:], OP.mult)
                    bp = sc2.tile([128, KW, WIN_C], BF16, name="bp", tag="bp")
                    nc.vector.tensor_tensor(bp[:], ikrow[1][:, ch], tch[1][:], OP.mult)
                    nc.vector.tensor_tensor(bkd, bkd, bp[:], OP.add)
                    nc.vector.tensor_tensor(bp[:], ikrow[2][:, ch], tch[2][:], OP.mult)
                    nc.vector.tensor_tensor(bkd, bkd, bp[:], OP.add)
                    bks.append(bk)
                    emit_tbox(1 + ch, bk)

                # ak = keep9*S - sum mu_ch * bk_ch
                ak = akbk.tile([128, 6, WIN_C], BF16, name="ak", tag="ak")
                akd = ak[:, 1:5, :]
                nc.vector.tensor_tensor(akd, keep9[:, 1:5, :], s16[:, 0], OP.mult)
                for ch in range(3):
                    am = sc2.tile([128, KW, WIN_C], BF16, name="am", tag="am")
                    nc.vector.tensor_tensor(am[:], mu_all[:, ch], bks[ch][:, 1:5, :], OP.mult)
                    nc.vector.tensor_tensor(akd, akd, am[:], OP.subtract)
                emit_tbox(0, ak)

                # residual: B-terms first (their copies land earliest), A last
                rps = []
                for ch in range(3):
                    rpp = sc2.tile([128, KP, OUT_C], F16, name=f"rpp_{ch}", tag=f"rpp_{ch}")
                    nc.gpsimd.tensor_tensor(rpp[:], imgn16[ch][:, :, 2:2 + OUT_C],
                                            ab16[1 + ch][:], OP.mult)
                    rps.append(rpp)
                r = sc2.tile([128, KP, OUT_C], F16, name="r", tag="r")
                nc.vector.tensor_tensor(r[:], o16[:, :, 2:2 + OUT_C], nkc16[:], OP.mult)
                rp = sc2.tile([128, KP, OUT_C], F16, name="rp", tag="rp")
                nc.vector.scalar_tensor_tensor(rp[:], tf16[:], float(c + 1), c100[:],
                                               OP.is_equal, OP.mult)
                nc.vector.tensor_tensor(r[:], r[:], rp[:], OP.add)
                for ch in range(3):
                    nc.vector.tensor_tensor(r[:], r[:], rps[ch][:], OP.subtract)
                nc.vector.tensor_tensor(r[:], r[:], ab16[0][:], OP.subtract)
                acc_c = sc2.tile([128, 1], F32, name="acc_c", tag="acc_c")
                nc.scalar.activation(sqd_dummy(c), r[:], ACTF.Square,
                                     bias=0.0, scale=1.0, accum_out=acc_c[:])
                nc.vector.tensor_tensor(acc_tot[:], acc_tot[:], acc_c[:], OP.add)

                if c + 2 < N_CLASSES:
                    front(c + 2)

            # ---- final reduce over partitions ----
            fin_ps = psb.tile([128, 3, PIX_C], F32, name="fin", tag="tb")
            nc.tensor.matmul(fin_ps[0:1, 0:1, 0:1], acc_tot[:], ones[:],
                             start=True, stop=True)
            fin = cst.tile([1, 1], F32, name="fin_sb", tag="fin_sb")
            nc.vector.tensor_copy(fin[:], fin_ps[0:1, 0:1, 0:1])
            nc.sync.dma_start(part_d[:], fin[:])

    nc.compile()
    return nc


def _get_program():
    global _PROGRAM
    if _PROGRAM is None:
        _PROGRAM = _build_program()
    return _PROGRAM


# ---------------- host-side prep ----------------

def _chunk_rows(arr, n_chunks, row_base=0):
    """arr [..., R, C] global rows -> [..., 128, n_chunks, C] with row
    128k+p placed at (p, k); row_base shifts arr's row 0 to chunk row_base*128."""
    lead = arr.shape[:-2]
    rr, cc = arr.shape[-2], arr.shape[-1]
    out = np.zeros(lead + (n_chunks * 128, cc), arr.dtype)
    out[..., row_base * 128:row_base * 128 + rr, :] = arr
    out = out.reshape(lead + (n_chunks, 128, cc))
    return np.moveaxis(out, -3, -2)  # [..., 128, n_chunks, cc]


def _col_slice(arr, c0, width):
    lead = arr.shape[:-1]
    out = np.zeros(lead + (width,), arr.dtype)
    alo, ahi = max(c0, 0), min(c0 + width, arr.shape[-1])
    if ahi > alo:
        out[..., alo - c0:ahi - c0] = arr[..., alo:ahi]
    return out


def _band(n_src, n_dst, offs, rows=None):
    w = np.zeros((n_src, n_dst), np.float32)
    src_rows = range(n_src) if rows is None else rows
    for si, sr in enumerate(src_rows):
        for m in range(n_dst):
            if sr - m in offs:
                w[si, m] = 1.0
    return w


def _host_inputs(cprob, img_org, trimap):
    cprob = np.ascontiguousarray(cprob, dtype=np.float32)
    img_org = np.ascontiguousarray(img_org, dtype=np.float32)
    trimap_f = np.ascontiguousarray(trimap, dtype=np.float32)

    w0 = _band(128, 128, (0, 1, 2))
    w1 = _band(2, 128, (0, 1, 2), rows=[128, 129])
    wt0 = _band(128, 128, (0, -1, -2))          # wt0[m,p]=1 if 0<=p-m<=2
    wt1 = np.zeros((128, 128), np.float32)
    wt1[126, 0] = wt1[127, 0] = wt1[127, 1] = 1.0
    wd0 = _band(128, 128, (-1, 0, 1))
    wdu = np.zeros((128, 128), np.float32)
    wdu[0, 127] = 1.0
    wdd = np.zeros((128, 128), np.float32)
    wdd[127, 0] = 1.0
    ones = np.ones((128, 1), np.float32)
    bf = ml_dtypes.bfloat16
    v9 = np.float32(np.float32(1.0 / 9.0).astype(bf))

    in_maps = []
    for core in range(8):
        b, q = core // 4, core % 4
        cb = 129 * q
        cpr = _chunk_rows(_col_slice(cprob[b], cb - 2, PIX_C), KP).astype(np.float16)
        img = _chunk_rows(_col_slice(np.moveaxis(img_org[b], -1, 0), cb - 2, PIX_C), KP).astype(np.float16)
        trm = _chunk_rows(_col_slice(trimap_f[b], cb - 3, T_C), 7, row_base=1).astype(np.float16)
        vm = np.zeros((128, KW, WIN_C), np.float32)
        rows_valid = (np.arange(128)[:, None] + 128 * np.arange(KW)[None, :]) < NW
        cols_valid = (cb - 2 + np.arange(WIN_C) >= 0) & (cb - 2 + np.arange(WIN_C) < NW)
        vm[rows_valid[:, :, None] & cols_valid[None, None, :]] = v9
        in_maps.append({
            "cpr": cpr, "img": img, "trim": trm,
            "vmask": vm.astype(bf),
            "w0f": w0.astype(np.float16), "w0b": w0.astype(bf),
            "w1f": w1.astype(np.float16), "w1b": w1.astype(bf),
            "wt0": wt0.astype(bf), "wt1": wt1.astype(bf),
            "wd0": wd0.astype(bf), "wdu": wdu.astype(bf), "wdd": wdd.astype(bf),
            "ones": ones,
        })
    return in_maps


def run(cprob, img_org, trimap, trace=False):
    nc = _get_program()
    in_maps = _host_inputs(cprob, img_org, trimap)
    res = run_bass_kernel_spmd(nc, in_maps, list(range(8)), trace=trace)
    total = sum(float(r["partial"][0, 0]) for r in res.results)
    out = np.float32(total / (float(N) * float(N)))
    return out, res


def kernel(cprob, img_org, trimap):
    out, _ = run(cprob, img_org, trimap)
    return out
